# revision 2
# baseline (speedup 1.0000x reference)
"""BibdLinear Trainium2 kernel: out = input @ (weight * mask).T

Shapes (hardcoded): input [8192, 4096] f32, weight [4096, 4096] f32,
mask [4096, 4096] f32 -> out [8192, 4096] f32.

Sharding (column-parallel x batch-parallel, 8 cores): 2 batch shards x
4 output-feature shards. Core c handles batch rows [(c//4)*4096, +4096)
and output features [(c%4)*1024, +1024); the host concatenates the 8
output slices.

Per-core device GEMM (Bass/Tile), K=4096 contraction split by dtype:
  - k-tiles 0..17 (K0=18): bf16 operands (x*SX, w*SW planes).
  - k-tiles 18..31 (KQ=14): fp8e4 DoubleRow "hi/lo pair" matmuls:
      stationary pair (x_hi, x_lo*SL), moving pair (w~, w~/SL)
      => psum += x_hi*w~ + x_lo*w~, i.e. x at ~17-bit effective
      precision and only w's single e4m3 quantization (~2.4% rms)
      touching the fp8 fraction. DoubleRow streams at 2x the bf16
      matmul rate, so these k-tiles cost half.
  End-to-end rel err ~1.76e-2 vs the f32 reference.
  All planes carry a uniform SX*SW scale, descaled at PSUM eviction
  (DVE tensor_scalar_mul / ACT activation-Copy with scale).

Schedule per core: 16 batch blocks of 256 rows; per block 32 k-tiles x
(2 batch subtiles x 4 feature chunks of N=256) accumulate into 8 PSUM
banks. Each accumulator is evicted immediately after its final matmul
(DVE/ACT alternating, staggered so the next block's matmuls never wait
on a bank); a bf16 k-tile is ordered last to widen the eviction window.
Weights live resident in SBUF; x arrives as per-block k-group strips
with 512B-contiguous descriptors; DMA k-groups are size-graded (small
first so the first matmul issues ~2us in, large after to respect the
~630ns/DMA HWDGE op rate). Outputs store as bf16 on the SWDGE queue
and the host upcasts to f32.
"""
import numpy as np
import ml_dtypes

import concourse.mybir as mybir
import concourse.tile as tile
from concourse import bacc

# ---------------------------------------------------------------- problem
BATCH, IN_F, OUT_F = 8192, 4096, 4096
B_S, O_S = 2, 4
B, OF = BATCH // B_S, OUT_F // O_S     # 4096, 1024 per core
N_CORES = 8

K0 = 18                                 # bf16 k-tiles
KQ = 32 - K0                            # fp8 DoubleRow k-tiles
KL = K0 * 128
SX, SW, SL = 16.0, 64.0, 32.0

NF = 256                                # matmul moving width
F32 = mybir.dt.float32
BF16 = mybir.dt.bfloat16
FP8 = mybir.dt.float8e4
NP_BF16 = ml_dtypes.bfloat16
NP_E4 = ml_dtypes.float8_e4m3

WL_GROUPS = [2, 2, 3, 3, 3, 4, 1]      # bf16 w k-groups (sum K0)
WQ_GROUPS = [4, 5, 5]                  # fp8 w k-groups (sum KQ)
XL_GROUPS0 = [2, 4, 6, 6]              # block0 bf16 x split
XQ_GROUPS0 = [7, 7]                    # block0 fp8 x split
XL_GROUPS = [6, 6, 6]                  # steady bf16 x
XQ_GROUPS = [7, 7]                     # steady fp8 x

_NC_CACHE = {}


# ---------------------------------------------------------- device program
def build_nc(iters=1, x_bufs=3, out_bufs=4):
    K = IN_F
    KO = K // 128                      # 32
    OC = OF // NF                      # 4
    NBLK = B // 256                    # 16
    SCL = 1.0 / (SX * SW)

    nc = bacc.Bacc(None, target_bir_lowering=False)

    xl = nc.dram_tensor("xl", [KL, B], BF16, kind="ExternalInput")
    xq = nc.dram_tensor("xq", [KQ * 128, B // 256, 2, 256], FP8,
                        kind="ExternalInput")
    wl = nc.dram_tensor("wl", [KL, OF], BF16, kind="ExternalInput")
    wq = nc.dram_tensor("wq", [KQ * 128, 2, OF], FP8, kind="ExternalInput")
    out = nc.dram_tensor("out", [B, OF], BF16, kind="ExternalOutput")

    xlPK = xl.rearrange("(ko p) b -> p ko b", p=128)
    xqPK = xq.rearrange("(kq p) c t b -> p kq c t b", p=128)
    wlPK = wl.rearrange("(ko p) o -> p ko o", p=128)
    wqPK = wq.rearrange("(kq p) t o -> p kq t o", p=128)

    with tile.TileContext(nc) as tc:
        with (
            tc.tile_pool(name="wpool", bufs=1) as wpool,
            tc.tile_pool(name="xpool", bufs=x_bufs) as xpool,
            tc.tile_pool(name="x0pool", bufs=1) as x0pool,
            tc.tile_pool(name="opool", bufs=out_bufs) as opool,
            tc.tile_pool(name="psum", bufs=1, space="PSUM") as psum_pool,
        ):
            for it in range(iters):
                wkl = [None] * K0
                wkq = [None] * KQ

                def load_wl_group(k0, sz):
                    wt = wpool.tile([128, sz, OF], BF16, tag=f"wl{k0}",
                                    name=f"wl{k0}_{it}")
                    nc.scalar.dma_start(wt, wlPK[:, k0:k0 + sz, :])
                    for j in range(sz):
                        wkl[k0 + j] = (wt, j)

                def load_wq_group(k0, sz):
                    wt = wpool.tile([128, sz, 2, OF], FP8, tag=f"wq{k0}",
                                    name=f"wq{k0}_{it}")
                    nc.scalar.dma_start(wt, wqPK[:, k0:k0 + sz, :, :])
                    for j in range(sz):
                        wkq[k0 + j] = (wt, j)

                def load_x_groups(blk, groups_l, groups_q, pool, tp):
                    xkl = [None] * K0
                    xkq = [None] * KQ
                    k0 = 0
                    for gi, sz in enumerate(groups_l):
                        xt = pool.tile([128, sz, 256], BF16, tag=f"{tp}l{gi}",
                                       name=f"{tp}l{gi}_{blk}_{it}")
                        nc.sync.dma_start(
                            xt, xlPK[:, k0:k0 + sz,
                                     blk * 256:(blk + 1) * 256])
                        for j in range(sz):
                            xkl[k0 + j] = (xt, j)
                        k0 += sz
                    k0 = 0
                    for gi, sz in enumerate(groups_q):
                        xt = pool.tile([128, sz, 2, 256], FP8,
                                       tag=f"{tp}q{gi}",
                                       name=f"{tp}q{gi}_{blk}_{it}")
                        nc.sync.dma_start(xt, xqPK[:, k0:k0 + sz, blk, :, :])
                        for j in range(sz):
                            xkq[k0 + j] = (xt, j)
                        k0 += sz
                    return xkl, xkq

                # startup: x block0 first, w graded in k order; the w group
                # holding the last-visited k-tile (K0-1) goes last
                xs = load_x_groups(0, XL_GROUPS0, XQ_GROUPS0, x0pool, "xa")
                for gi, sz in enumerate(WL_GROUPS[:-1]):
                    load_wl_group(sum(WL_GROUPS[:gi]), sz)
                for gi, sz in enumerate(WQ_GROUPS):
                    load_wq_group(sum(WQ_GROUPS[:gi]), sz)
                load_wl_group(sum(WL_GROUPS[:-1]), WL_GROUPS[-1])

                def evict(i, psums, ots):
                    bs, oc = divmod(i, OC)
                    dst = ots[bs][:, oc * NF:(oc + 1) * NF]
                    if i % 2:
                        nc.scalar.activation(
                            dst, psums[i],
                            mybir.ActivationFunctionType.Copy, scale=SCL)
                    else:
                        nc.vector.tensor_scalar_mul(dst, psums[i], SCL)

                for blk in range(NBLK):
                    if blk > 0:
                        xs = xs_next
                    if blk + 1 < NBLK:
                        xs_next = load_x_groups(blk + 1, XL_GROUPS,
                                                XQ_GROUPS, xpool, "x")
                    xkl, xkq = xs
                    psums = [
                        psum_pool.tile([128, NF], F32, tag=f"ps{i}",
                                       name=f"ps{i}_{blk}_{it}")
                        for i in range(8)
                    ]
                    ots = [opool.tile([128, OF], BF16, tag=f"ot{bs}",
                                      name=f"ot{bs}_{blk}_{it}")
                           for bs in range(2)]
                    # visit a bf16 k-tile last: its 107ns/mm tail leaves the
                    # staggered evictions time to clear the banks
                    korder = (list(range(K0 - 1)) + list(range(K0, KO))
                              + [K0 - 1])
                    for ki, k in enumerate(korder):
                        first, lastk = ki == 0, ki == KO - 1
                        for bs in range(2):
                            if k < K0:
                                xt, xj = xkl[k]
                                lhsT = xt[:, xj, bs * 128:(bs + 1) * 128]
                                wt, wj = wkl[k]
                                for oc in range(OC):
                                    nc.tensor.matmul(
                                        psums[bs * OC + oc], lhsT,
                                        wt[:, wj, oc * NF:(oc + 1) * NF],
                                        start=first, stop=lastk,
                                    )
                            else:
                                xt, xj = xkq[k - K0]
                                lhsT = xt[:, xj, :, bs * 128:(bs + 1) * 128]
                                wt, wj = wkq[k - K0]
                                for oc in range(OC):
                                    nc.tensor.matmul(
                                        psums[bs * OC + oc], lhsT,
                                        wt[:, wj, :, oc * NF:(oc + 1) * NF],
                                        start=first, stop=lastk,
                                        perf_mode=(
                                            mybir.MatmulPerfMode.DoubleRow),
                                    )
                    last = blk == NBLK - 1
                    if not last:
                        for i in range(8):
                            evict(i, psums, ots)
                        for bs in range(2):
                            nc.gpsimd.dma_start(
                                out[(blk * 2 + bs) * 128:
                                    (blk * 2 + bs + 1) * 128, :], ots[bs])
                    else:
                        # tail: store finished oc-pairs promptly, spread
                        # across queues
                        chunk_q = [nc.sync, nc.scalar, nc.gpsimd, nc.scalar]
                        for i in range(8):
                            evict(i, psums, ots)
                            if i % 2 == 1:
                                bs, oc = divmod(i, OC)
                                row = (blk * 2 + bs) * 128
                                c0 = (oc - 1) * NF
                                chunk_q[(i - 1) // 2].dma_start(
                                    out[row:row + 128, c0:c0 + 2 * NF],
                                    ots[bs][:, c0:c0 + 2 * NF])

    nc.compile()
    return nc


def _get_nc():
    if "nc" not in _NC_CACHE:
        _NC_CACHE["nc"] = build_nc()
    return _NC_CACHE["nc"]


# ------------------------------------------------------------- host prep
def _prep_x(xs):
    """xs [B, 4096] f32 (batch shard) -> xl bf16, xq packed fp8 pairs."""
    xt = np.ascontiguousarray(xs.T) * SX           # [4096, B] scaled
    xl = xt[:KL].astype(NP_BF16)
    q = xt[KL:]
    hi = q.astype(NP_E4)
    lo = ((q - hi.astype(np.float32)) * SL).astype(NP_E4)
    pair = np.stack([hi, lo], axis=1)              # [KQ*128, 2, B]
    xq = np.ascontiguousarray(
        pair.reshape(KQ * 128, 2, B // 256, 256).transpose(0, 2, 1, 3))
    return xl, xq


def _prep_w(ws):
    """ws [OF, 4096] f32 (masked weight shard) -> wl bf16, wq fp8 pairs."""
    wt = np.ascontiguousarray(ws.T) * SW           # [4096, OF] scaled
    wl = wt[:KL].astype(NP_BF16)
    q = wt[KL:]
    hi = q.astype(NP_E4)
    lo = (q / SL).astype(NP_E4)
    wq = np.ascontiguousarray(np.stack([hi, lo], axis=1))
    return wl, wq


def shard_inputs(input, weight, mask):
    x = np.asarray(input, dtype=np.float32)
    s = np.asarray(weight, dtype=np.float32) * np.asarray(mask,
                                                          dtype=np.float32)
    xparts = [_prep_x(x[i * B:(i + 1) * B]) for i in range(B_S)]
    wparts = [_prep_w(s[j * OF:(j + 1) * OF]) for j in range(O_S)]
    in_maps = []
    for c in range(N_CORES):
        xl, xq = xparts[c // O_S]
        wl, wq = wparts[c % O_S]
        in_maps.append({"xl": xl, "xq": xq, "wl": wl, "wq": wq})
    return in_maps


def gather_output(results):
    outp = np.empty((BATCH, OUT_F), np.float32)
    for c in range(N_CORES):
        b0 = (c // O_S) * B
        o0 = (c % O_S) * OF
        outp[b0:b0 + B, o0:o0 + OF] = results[c]["out"].astype(np.float32)
    return outp


def kernel(input, weight, mask):
    from concourse.bass_utils import run_bass_kernel_spmd
    in_maps = shard_inputs(input, weight, mask)
    res = run_bass_kernel_spmd(_get_nc(), in_maps,
                               core_ids=list(range(N_CORES)))
    return gather_output(res.results)
